# revision 1
# baseline (speedup 1.0000x reference)
"""EGT layer kernel for Trainium2, data-parallel over the batch dim on 8 NeuronCores.

Sharding: batch B=16 is split 2-per-core across the 8 cores (data parallel,
per the sharding hint). Each core computes the full EGT layer for its two
graphs; outputs are gathered back to a single full-shape result.
"""

import numpy as np
import jax
import jax.numpy as jnp
from functools import partial

B, L, D, EW, H = 16, 256, 512, 64, 32
DD = D // H
CLIP_MIN, CLIP_MAX = -5.0, 5.0
LN_EPS = 1e-5
N_CORES = 8
B_LOC = B // N_CORES

WEIGHT_NAMES = [
    "ln_h_g", "ln_h_b", "ln_e_g", "ln_e_b", "W_QKV", "b_QKV", "W_E", "b_E",
    "W_G", "b_G", "W_Oh", "b_Oh", "ffn_ln_h_g", "ffn_ln_h_b", "W_h1", "b_h1",
    "W_h2", "b_h2", "W_Oe", "b_Oe", "ffn_ln_e_g", "ffn_ln_e_b", "W_e1",
    "b_e1", "W_e2", "b_e2",
]


def _ln(x, g, b):
    m = jnp.mean(x, axis=-1, keepdims=True)
    v = jnp.mean(jnp.square(x - m), axis=-1, keepdims=True)
    return (x - m) * jax.lax.rsqrt(v + LN_EPS) * g + b


def _egt_shard(h, e, mask, w):
    """One core's shard: h [B_LOC,L,D], e [B_LOC,L,L,EW], mask [B_LOC,L,L,1]."""
    h_r1, e_r1 = h, e
    h_ln = _ln(h, w["ln_h_g"], w["ln_h_b"])
    e_ln = _ln(e, w["ln_e_g"], w["ln_e_b"])
    QKV = h_ln @ w["W_QKV"] + w["b_QKV"]
    E = e_ln @ w["W_E"] + w["b_E"]
    G = e_ln @ w["W_G"] + w["b_G"]
    qkv = QKV.reshape(B_LOC, L, 3 * DD, H)
    Q, K, V = qkv[:, :, :DD], qkv[:, :, DD:2 * DD], qkv[:, :, 2 * DD:]
    A_hat = jnp.einsum("bldh,bmdh->blmh", Q, K) * (DD ** -0.5)
    H_hat = jnp.clip(A_hat, CLIP_MIN, CLIP_MAX) + E
    gates = jax.nn.sigmoid(G + mask)
    A_tild = jax.nn.softmax(H_hat + mask, axis=2) * gates
    V_att = jnp.einsum("blmh,bmkh->blkh", A_tild, V).reshape(B_LOC, L, D)
    h = V_att @ w["W_Oh"] + w["b_Oh"] + h_r1
    h_r2 = h
    h = jax.nn.elu(_ln(h, w["ffn_ln_h_g"], w["ffn_ln_h_b"]) @ w["W_h1"]
                   + w["b_h1"]) @ w["W_h2"] + w["b_h2"] + h_r2
    e = H_hat @ w["W_Oe"] + w["b_Oe"] + e_r1
    e_r2 = e
    e = jax.nn.elu(_ln(e, w["ffn_ln_e_g"], w["ffn_ln_e_b"]) @ w["W_e1"]
                   + w["b_e1"]) @ w["W_e2"] + w["b_e2"] + e_r2
    return h, e


_egt_pmapped = jax.pmap(_egt_shard, in_axes=(0, 0, 0, None), out_axes=0)


def kernel(**inputs):
    h = np.asarray(inputs["h"], dtype=np.float32)
    e = np.asarray(inputs["e"], dtype=np.float32)
    mask = np.asarray(inputs["mask"], dtype=np.float32)
    w = {k: jnp.asarray(np.asarray(inputs[k], dtype=np.float32))
         for k in WEIGHT_NAMES}

    h_sh = h.reshape(N_CORES, B_LOC, L, D)
    e_sh = e.reshape(N_CORES, B_LOC, L, L, EW)
    m_sh = mask.reshape(N_CORES, B_LOC, L, L, 1)

    h_out, e_out = _egt_pmapped(h_sh, e_sh, m_sh, w)
    h_out = np.asarray(h_out).reshape(B, L, D)
    e_out = np.asarray(e_out).reshape(B, L, L, EW)
    return h_out, e_out


# revision 2
# speedup vs baseline: 119.4693x; 119.4693x over previous
"""EGT layer kernel for Trainium2, data-parallel over the batch dim on 8 NeuronCores.

Sharding: batch B=16 is split 2-per-core across the 8 cores (data parallel,
per the sharding hint). Each core computes the full EGT layer for its two
graphs; outputs are gathered back to a single full-shape result.

The L x L x H logits/gates tensors (the fusion target) and the edge FFN are
computed with bf16 matmul inputs and fp32 accumulation: every bf16 product
feeds either a softmax/sigmoid (error-tolerant) or a small additive term on
top of an fp32 residual path, so the end-to-end error stays ~1e-4 while the
dominant matmul/data volume halves.
"""

import numpy as np
import jax
import jax.numpy as jnp

B, L, D, EW, H = 16, 256, 512, 64, 32
DD = D // H
CLIP_MIN, CLIP_MAX = -5.0, 5.0
LN_EPS = 1e-5
N_CORES = 8
B_LOC = B // N_CORES

bf16 = jnp.bfloat16
f32 = jnp.float32

WEIGHT_NAMES = [
    "ln_h_g", "ln_h_b", "ln_e_g", "ln_e_b", "W_QKV", "b_QKV", "W_E", "b_E",
    "W_G", "b_G", "W_Oh", "b_Oh", "ffn_ln_h_g", "ffn_ln_h_b", "W_h1", "b_h1",
    "W_h2", "b_h2", "W_Oe", "b_Oe", "ffn_ln_e_g", "ffn_ln_e_b", "W_e1",
    "b_e1", "W_e2", "b_e2",
]


def _ln(x, g, b):
    m = jnp.mean(x, axis=-1, keepdims=True)
    v = jnp.mean(jnp.square(x - m), axis=-1, keepdims=True)
    return (x - m) * jax.lax.rsqrt(v + LN_EPS) * g + b


def _mmb(a, w):
    """Matmul with bf16 inputs, fp32 accumulation."""
    return jnp.matmul(a.astype(bf16), w.astype(bf16), preferred_element_type=f32)


def _egt_shard(h, e, mask, w):
    """One core's shard: h [B_LOC,L,D], e [B_LOC,L,L,EW], mask [B_LOC,L,L,1]."""
    h_r1, e_r1 = h, e
    h_ln = _ln(h, w["ln_h_g"], w["ln_h_b"])
    e_ln = _ln(e, w["ln_e_g"], w["ln_e_b"])
    QKV = h_ln @ w["W_QKV"] + w["b_QKV"]
    EG = _mmb(e_ln, jnp.concatenate([w["W_E"], w["W_G"]], axis=1))
    E = EG[..., :H] + w["b_E"]
    G = EG[..., H:] + w["b_G"]
    qkv = QKV.reshape(B_LOC, L, 3 * DD, H)
    Q, K, V = qkv[:, :, :DD], qkv[:, :, DD:2 * DD], qkv[:, :, 2 * DD:]
    A_hat = jnp.einsum("bldh,bmdh->blmh", Q.astype(bf16), K.astype(bf16),
                       preferred_element_type=f32) * (DD ** -0.5)
    H_hat = jnp.clip(A_hat, CLIP_MIN, CLIP_MAX) + E
    gates = jax.nn.sigmoid(G + mask)
    A_tild = jax.nn.softmax(H_hat + mask, axis=2) * gates
    V_att = jnp.einsum("blmh,bmkh->blkh", A_tild.astype(bf16), V.astype(bf16),
                       preferred_element_type=f32).reshape(B_LOC, L, D)
    h = V_att @ w["W_Oh"] + w["b_Oh"] + h_r1
    h_r2 = h
    h = jax.nn.elu(_ln(h, w["ffn_ln_h_g"], w["ffn_ln_h_b"]) @ w["W_h1"]
                   + w["b_h1"]) @ w["W_h2"] + w["b_h2"] + h_r2
    e = _mmb(H_hat, w["W_Oe"]) + w["b_Oe"] + e_r1
    e_r2 = e
    e = _mmb(jax.nn.elu(_mmb(_ln(e, w["ffn_ln_e_g"], w["ffn_ln_e_b"]),
                             w["W_e1"]) + w["b_e1"]),
             w["W_e2"]) + w["b_e2"] + e_r2
    return h, e


_egt_pmapped = jax.pmap(_egt_shard, in_axes=(0, 0, 0, None), out_axes=0)


def kernel(**inputs):
    h = np.asarray(inputs["h"], dtype=np.float32)
    e = np.asarray(inputs["e"], dtype=np.float32)
    mask = np.asarray(inputs["mask"], dtype=np.float32)
    w = {k: jnp.asarray(np.asarray(inputs[k], dtype=np.float32))
         for k in WEIGHT_NAMES}

    h_sh = h.reshape(N_CORES, B_LOC, L, D)
    e_sh = e.reshape(N_CORES, B_LOC, L, L, EW)
    m_sh = mask.reshape(N_CORES, B_LOC, L, L, 1)

    h_out, e_out = _egt_pmapped(h_sh, e_sh, m_sh, w)
    h_out = np.asarray(h_out).reshape(B, L, D)
    e_out = np.asarray(e_out).reshape(B, L, L, EW)
    return h_out, e_out
